# revision 36
# baseline (speedup 1.0000x reference)
"""Multi-head attention (RoPE + causal softmax + out-proj) on 8 TRN2 NeuronCores.

Sharding: core c handles batch b = c // 2 and head-half g = c % 2 (8 of 16
heads). Each core computes q/k/v projections for its heads, RoPE, causal
attention, and a partial transposed output projection
outT = (y_heads @ Wo_part.T).T; the host sums the two partials per batch.

Device layout notes (v2):
 - All matmul operands are bf16 (PSUM accumulation stays fp32); host converts
   x / Wqkv / Wo to bf16, halving input DMA and enabling 1-cycle/row matmuls
   at any moving width.
 - q/k weight columns are permuted host-side into pair-interleaved layout:
   each 128-row projection group = 2 heads x [even dims (32) | odd dims (32)].
   RoPE runs as full-width vector ops using a PE permutation matmul (swap of
   32-row blocks) to produce the partner operand, with a sign-folded sin tile.
 - Attention is k-major with K=64 stationary per head (single matmul per
   (head, k-tile)); q-chunks are 256 wide (finer causal granularity).
   Causal masking is a -1e30 bias accumulated into sT via an identity-
   stationary matmul on the diagonal tiles, so exp -> y has no mask step.
 - exp on ScalarE (1/sqrt(dh) folded into the activation scale) writes bf16;
   sT PSUM is double-buffered and emission is software-pipelined as
   s(kt) -> exp(kt) -> y(kt-1) so the PE never idles on the scalar engine.
 - v_aug carries a ones column per head, making row 64 of yT the softmax
   denominator; normalization (reciprocal + partition broadcast + multiply)
   rides the DVE/GpSimd/DMA queues and never blocks the PE.
"""

import numpy as np
import ml_dtypes

BF16 = ml_dtypes.bfloat16

B, T, C, H = 4, 2048, 1024, 16
DH = C // H  # 64
NCORES = 8
HPC = H // 2  # 8 heads per core
QR = HPC * DH  # 512 rows per q/k/v section
TS = 512  # projection t-chunk width
NTS = T // TS  # 4
CC = C // 128  # 8 contraction chunks
QS = 512  # attention q-chunk width
NQS = T // QS  # 4
NKT = T // 128  # 16 k-tiles

_CACHE = {}


def _build_program():
    import concourse.mybir as mybir
    import concourse.tile as tile
    from concourse import bacc

    f32 = mybir.dt.float32
    f32r = mybir.dt.float32r
    bf16 = mybir.dt.bfloat16
    EXP = mybir.ActivationFunctionType.Exp

    nc = bacc.Bacc(trn_type="TRN2")

    xT = nc.dram_tensor("xT", [C, T], bf16, kind="ExternalInput").ap()
    wqkp = nc.dram_tensor("wqkp", [128, 8 * C], bf16, kind="ExternalInput").ap()
    wvT = nc.dram_tensor("wvT", [C, QR], bf16, kind="ExternalInput").ap()
    woT = nc.dram_tensor("woT", [QR, C], bf16, kind="ExternalInput").ap()
    cosT = nc.dram_tensor("cosT", [128, T], f32, kind="ExternalInput").ap()
    sinT = nc.dram_tensor("sinT", [128, T], f32, kind="ExternalInput").ap()
    maskb = nc.dram_tensor("maskb", [128, 128], f32, kind="ExternalInput").ap()
    p32 = nc.dram_tensor("p32", [128, 128], f32, kind="ExternalInput").ap()
    outT = nc.dram_tensor("outT", [C, T], bf16, kind="ExternalOutput").ap()

    with tile.TileContext(nc) as tc:
        with tc.tile_pool(name="persist", bufs=1) as pp:
            # rope'd q/k pair tiles: qk[part][pr][ts] = [128, TS] bf16, rows
            # = head 2*pr: [even dims 32 | odd dims 32], head 2*pr+1 likewise.
            # Per-ts tiles keep phase-B dependencies fine-grained.
            qk = [
                [
                    [
                        pp.tile(
                            [128, TS],
                            bf16,
                            tag=f"qk{part}{pr}{ts}",
                            name=f"qk{part}{pr}{ts}",
                        )
                        for ts in range(NTS)
                    ]
                    for pr in range(4)
                ]
                for part in range(2)
            ]
            # v with a ones column per head: [t-chunk 128, 8 * 65] bf16
            v_aug = [
                pp.tile([128, HPC * 65], bf16, tag=f"va{t}", name=f"va{t}")
                for t in range(NKT)
            ]
            # out-proj weights + y accumulator tiles (persistent, written in B)
            wot = [
                pp.tile([128, C], bf16, tag=f"wo{cc}", name=f"wo{cc}")
                for cc in range(4)
            ]
            yT_all = [
                pp.tile([128, T], bf16, tag=f"ya{j}", name=f"ya{j}") for j in range(4)
            ]
            mt = pp.tile([128, 128], f32, tag="maskb")
            psw = pp.tile([128, 128], f32r, tag="p32")
            ct = pp.tile([128, T], f32, tag="cos")
            st_ = pp.tile([128, T], f32, tag="sin")


            # ---------------- phase A: qkv projection + rope ----------------
            with (
                tc.tile_pool(name="wpool", bufs=1) as wp,
                tc.tile_pool(name="xpool", bufs=16) as xp,
                tc.tile_pool(name="pstage", bufs=4) as sp,
                tc.tile_pool(name="ropetmp", bufs=6) as rt,
                tc.tile_pool(name="psA", bufs=4, space="PSUM") as psA,
                tc.tile_pool(name="psV", bufs=2, space="PSUM") as psV,
                tc.tile_pool(name="psB", bufs=2, space="PSUM") as psB,
            ):
                # first q/k weight col-group (cc-packed), then the first x
                # chunk set, then the rest — so the first matmul group isn't
                # stuck behind the full weights
                wqk = [None] * 8
                w = wp.tile([128, C], bf16, tag="wg0", name="wg0")
                nc.sync.dma_start(w[:], wqkp[:, 0:C])
                wqk[0] = w
                xts0 = []
                for cc in range(CC):
                    xt = xp.tile([128, TS], bf16, tag="xts", name=f"x0{cc}")
                    nc.sync.dma_start(xt[:], xT[128 * cc : 128 * (cc + 1), 0:TS])
                    xts0.append(xt)
                for g in range(1, 8):
                    w = wp.tile([128, C], bf16, tag=f"wg{g}", name=f"wg{g}")
                    nc.sync.dma_start(w[:], wqkp[:, C * g : C * (g + 1)])
                    wqk[g] = w
                wv = [None] * CC
                for cc in range(CC):
                    w = wp.tile([128, QR], bf16, tag=f"wv{cc}", name=f"wv{cc}")
                    nc.sync.dma_start(w[:], wvT[128 * cc : 128 * (cc + 1), :])
                    wv[cc] = w
                nc.sync.dma_start(ct[:], cosT[:])
                nc.sync.dma_start(st_[:], sinT[:])
                nc.sync.dma_start(mt[:], maskb[:])
                nc.sync.dma_start(psw[:], p32[:].bitcast(f32r))
                for cc in range(4):
                    nc.sync.dma_start(
                        wot[cc][:], woT[128 * cc : 128 * (cc + 1), :]
                    )

                for t in range(NKT):
                    var = v_aug[t].rearrange("p (h d) -> p h d", h=HPC)
                    nc.gpsimd.memset(var[:, :, 64:65], 1.0)

                for ts in range(NTS):
                    if ts == 0:
                        xts = xts0
                    else:
                        xts = []
                        for cc in range(CC):
                            xt = xp.tile([128, TS], bf16, tag="xts", name=f"x{ts}{cc}")
                            nc.sync.dma_start(
                                xt[:],
                                xT[128 * cc : 128 * (cc + 1), TS * ts : TS * (ts + 1)],
                            )
                            xts.append(xt)

                    csl = ct[:, TS * ts : TS * (ts + 1)]
                    ssl = st_[:, TS * ts : TS * (ts + 1)]
                    for part in range(2):  # 0=q, 1=k
                        for pr in range(4):  # pair of heads (2pr, 2pr+1)
                            p = psA.tile(
                                [128, TS], f32, tag="proj", name=f"p{ts}{part}{pr}"
                            )
                            wg = wqk[4 * part + pr]
                            for cc in range(CC):
                                nc.tensor.matmul(
                                    p[:],
                                    wg[:, 128 * cc : 128 * (cc + 1)],
                                    xts[cc][:],
                                    start=(cc == 0),
                                    stop=(cc == CC - 1),
                                )
                            ps = sp.tile(
                                [128, TS], f32r, tag="ps", name=f"ps{ts}{part}{pr}"
                            )
                            nc.scalar.copy(ps[:], p[:])
                            # 32-row block swap (x1 <-> x2 per head) via a
                            # permutation matmul
                            pw = psB.tile(
                                [128, TS], f32, tag="pw", name=f"pw{ts}{part}{pr}"
                            )
                            nc.tensor.matmul(
                                pw[:], psw[:], ps[:], start=True, stop=True
                            )
                            t1 = rt.tile([128, TS], f32, tag="rt", name=f"t1{ts}{part}{pr}")
                            t2 = rt.tile([128, TS], f32, tag="rt", name=f"t2{ts}{part}{pr}")
                            nc.vector.tensor_mul(t1[:], ps[:], csl)
                            nc.vector.tensor_mul(t2[:], pw[:], ssl)
                            nc.vector.tensor_add(
                                qk[part][pr][ts][:], t1[:], t2[:]
                            )

                    # v projection: x chunk stationary -> p[t, v-features]
                    for tr4 in range(4):
                        t = 4 * ts + tr4
                        p = psV.tile([128, QR], f32, tag="pv", name=f"pv{ts}{tr4}")
                        for cc in range(CC):
                            nc.tensor.matmul(
                                p[:],
                                xts[cc][:, 128 * tr4 : 128 * (tr4 + 1)],
                                wv[cc][:],
                                start=(cc == 0),
                                stop=(cc == CC - 1),
                            )
                        var = v_aug[t].rearrange("p (h d) -> p h d", h=HPC)
                        nc.vector.tensor_copy(
                            var[:, :, 0:64],
                            p[:].rearrange("p (h d) -> p h d", h=HPC),
                        )

            # ---------------- phase B: attention ----------------
            with (
                tc.tile_pool(name="epool", bufs=4) as ep,
                tc.tile_pool(name="dstage", bufs=4) as dsp,
                tc.tile_pool(name="rstage", bufs=4) as rsp,
                tc.tile_pool(name="bstage", bufs=4) as bsp,
                tc.tile_pool(name="psS", bufs=2, space="PSUM") as psS,
                tc.tile_pool(name="psY", bufs=2, space="PSUM") as psY,
            ):
                for pr in range(4):  # head pairs: heads 2pr, 2pr+1
                    for qi in range(NQS):
                        q0 = QS * qi
                        nkt = 4 * (qi + 1)
                        yTs = [
                            psY.tile([65, QS], f32, tag=f"yT{i}", name=f"yT{pr}_{qi}_{i}")
                            for i in range(2)
                        ]
                        pend = None  # (kt, eT) awaiting y-matmuls
                        for kt in range(nkt):
                            k0 = 128 * kt
                            r = kt - 4 * qi
                            sT = psS.tile(
                                [128, 2 * QS], f32, tag="sT", name=f"sT{pr}_{qi}_{kt}"
                            )
                            c0 = 128 * max(r, 0)  # causally-valid col offset
                            tsk = kt // 4
                            kc = 128 * (kt % 4)
                            for lh in range(2):
                                prt0 = 64 * lh
                                nc.tensor.matmul(
                                    sT[:, QS * lh + c0 : QS * (lh + 1)],
                                    qk[1][pr][tsk][prt0 : prt0 + 64, kc : kc + 128],
                                    qk[0][pr][qi][prt0 : prt0 + 64, c0:QS],
                                    start=True,
                                    stop=True,
                                )
                            if r >= 0:
                                # -1e30 triangular bias on the 128-wide
                                # diagonal strip (both heads, one DVE op);
                                # the fully-masked prefix is never computed
                                sv0 = sT[:].rearrange("p (l q) -> p l q", l=2)
                                strip = sv0[:, :, c0 : c0 + 128]
                                nc.vector.tensor_add(
                                    strip,
                                    strip,
                                    mt[:].unsqueeze(1).broadcast_to([128, 2, 128]),
                                )
                            eT = ep.tile(
                                [128, 2 * QS], bf16, tag="eT", name=f"eT{pr}_{qi}_{kt}"
                            )
                            if r < 1:
                                nc.scalar.activation(eT[:], sT[:], EXP, scale=0.125)
                            else:
                                # exp only the causally-reachable suffix per
                                # head (strided AP covers both heads in one
                                # instruction); the masked prefix is never
                                # read downstream
                                ev = eT[:].rearrange("p (l q) -> p l q", l=2)
                                sv = sT[:].rearrange("p (l q) -> p l q", l=2)
                                nc.scalar.activation(
                                    ev[:, :, c0:QS],
                                    sv[:, :, c0:QS],
                                    EXP,
                                    scale=0.125,
                                )
                            if pend is not None:
                                pkt, peT, pc0 = pend
                                for lh in range(2):
                                    h = 2 * pr + lh
                                    nc.tensor.matmul(
                                        yTs[lh][:, pc0:QS],
                                        v_aug[pkt][:, 65 * h : 65 * h + 65],
                                        peT[:, QS * lh + pc0 : QS * (lh + 1)],
                                        start=(pkt == 0),
                                        stop=False,
                                    )
                            pend = (kt, eT, c0)
                        pkt, peT, pc0 = pend
                        for lh in range(2):
                            h = 2 * pr + lh
                            nc.tensor.matmul(
                                yTs[lh][:, pc0:QS],
                                v_aug[pkt][:, 65 * h : 65 * h + 65],
                                peT[:, QS * lh + pc0 : QS * (lh + 1)],
                                start=(pkt == 0),
                                stop=True,
                            )
                        # denominator staging -> reciprocal -> broadcast ->
                        # fused normalize-copy out of PSUM (bf16)
                        dn2 = dsp.tile([2, QS], f32, tag="dn", name=f"dn{pr}_{qi}")
                        for lh in range(2):
                            dtmp = dsp.tile(
                                [65, QS], f32, tag="dt", name=f"dt{pr}_{lh}_{qi}"
                            )
                            nc.vector.tensor_copy(dtmp[64:65, :], yTs[lh][64:65, :])
                            nc.sync.dma_start(dn2[lh : lh + 1, :], dtmp[64:65, :])
                        rcp2 = rsp.tile([2, QS], f32, tag="rcp", name=f"rcp{pr}_{qi}")
                        nc.vector.reciprocal_approx_fast(rcp2[:], dn2[:])
                        for lh in range(2):
                            rtile = rsp.tile(
                                [1, QS], f32, tag="rr", name=f"rr{pr}_{lh}_{qi}"
                            )
                            nc.sync.dma_start(rtile[:], rcp2[lh : lh + 1, :])
                            bcS = bsp.tile(
                                [128, QS], f32, tag="bb", name=f"bb{pr}_{lh}_{qi}"
                            )
                            nc.gpsimd.partition_broadcast(bcS[:], rtile[:])
                            nc.vector.tensor_mul(
                                yT_all[pr][64 * lh : 64 * lh + 64, q0 : q0 + QS],
                                yTs[lh][0:64, :],
                                bcS[64 * lh : 64 * lh + 64, :],
                            )

            # ---------------- phase C: out projection ----------------
            with (
                tc.tile_pool(name="ostage", bufs=4) as osp,
                tc.tile_pool(name="psW", bufs=4, space="PSUM") as psW,
            ):
                for ts in range(NTS):
                    for co in range(8):
                        p = psW.tile([128, TS], f32, tag="op", name=f"o{ts}{co}")
                        for cc in range(4):
                            nc.tensor.matmul(
                                p[:],
                                wot[cc][:, 128 * co : 128 * (co + 1)],
                                yT_all[cc][:, TS * ts : TS * (ts + 1)],
                                start=(cc == 0),
                                stop=(cc == 3),
                            )
                        o = osp.tile([128, TS], bf16, tag="os", name=f"os{ts}{co}")
                        nc.vector.tensor_copy(o[:], p[:])
                        nc.sync.dma_start(
                            outT[128 * co : 128 * (co + 1), TS * ts : TS * (ts + 1)],
                            o[:],
                        )

    nc.compile()
    return nc


def _get_program():
    if "nc" not in _CACHE:
        _CACHE["nc"] = _build_program()
    return _CACHE["nc"]


def _host_inputs(x, cos, sin, Wqkv, Wo):
    """Build the 8 per-core input maps."""
    # q/k head-section permutation (head-relative, 512 rows): pair-interleaved
    # [h0 even dims | h0 odd dims | h1 even | h1 odd | h2 even | ...]
    perm = []
    for lh in range(HPC):
        for par in range(2):
            for jj in range(32):
                perm.append(64 * lh + 2 * jj + par)
    perm = np.asarray(perm)

    cosT4 = np.ascontiguousarray(np.tile(cos.T, (4, 1)).astype(np.float32))
    sT = sin.T.astype(np.float32)
    sinT4 = np.ascontiguousarray(np.concatenate([-sT, sT, -sT, sT], axis=0))

    # causal bias tile [128, 128] f32: Tri[p, c] = -1e30 where c < p else 0
    maskb = np.ascontiguousarray(
        np.where(np.arange(128)[None, :] < np.arange(128)[:, None], -1e30, 0.0)
    ).astype(np.float32)

    # 32-block swap permutation: out[m] = in[m ^ 32]
    idx = np.arange(128)
    p32 = np.zeros((128, 128), dtype=np.float32)
    p32[idx ^ 32, idx] = 1.0

    in_maps = []
    for c in range(NCORES):
        b, g = c // 2, c % 2
        hs0 = HPC * g
        sec = np.arange(QR) + DH * hs0  # this core's rows within a section
        Wq = Wqkv[sec[perm], :]
        Wk = Wqkv[C + sec[perm], :]
        Wv = Wqkv[2 * C + sec, :]
        wqkT = np.concatenate([Wq, Wk], 0).T  # [C, 1024]
        # pack col-group-major, cc-contiguous: wqkp[:, 1024g+128cc : +128] =
        # wqkT[128cc : 128(cc+1), 128g : 128(g+1)]
        wqkp = np.empty((128, 8 * C), dtype=np.float32)
        for gg in range(8):
            for cc in range(CC):
                wqkp[:, C * gg + 128 * cc : C * gg + 128 * (cc + 1)] = wqkT[
                    128 * cc : 128 * (cc + 1), 128 * gg : 128 * (gg + 1)
                ]
        wvT = np.ascontiguousarray(Wv.T).astype(BF16)
        woTc = np.ascontiguousarray(Wo[:, sec].T).astype(BF16)
        xTb = np.ascontiguousarray(x[b].T).astype(BF16)
        in_maps.append(
            {
                "xT": xTb,
                "wqkp": np.ascontiguousarray(wqkp).astype(BF16),
                "wvT": wvT,
                "woT": woTc,
                "cosT": cosT4,
                "sinT": sinT4,
                "maskb": maskb,
                "p32": p32,
            }
        )
    return in_maps


def kernel(x, cos, sin, Wqkv, Wo, _want_profile=False):
    from concourse.bass_utils import run_bass_kernel_spmd

    x = np.asarray(x, dtype=np.float32)
    cos = np.asarray(cos, dtype=np.float32)
    sin = np.asarray(sin, dtype=np.float32)
    Wqkv = np.asarray(Wqkv, dtype=np.float32)
    Wo = np.asarray(Wo, dtype=np.float32)

    nc = _get_program()
    in_maps = _host_inputs(x, cos, sin, Wqkv, Wo)
    res = run_bass_kernel_spmd(nc, in_maps, list(range(NCORES)), trace=_want_profile)
    out = np.empty((B, T, C), dtype=np.float32)
    for b in range(B):
        acc = (
            res.results[2 * b]["outT"].astype(np.float32)
            + res.results[2 * b + 1]["outT"].astype(np.float32)
        )
        out[b] = acc.T
    if _want_profile:
        return out, res
    return out
